# revision 74
# baseline (speedup 1.0000x reference)
"""Contrastive patch loss (InfoNCE over sampled voxel patches) on 8 TRN2 NeuronCores.

Math
----
Reference computes, per patch p and batch b, cs[k,l] = <t2n[:,i_pk], t1n[:,i_pl]>
over k=512 sampled voxels (i = idx[p]), e = exp(cs/bw), then the mean over
(p,b,j) of -log(0.5*e_jj*(1/colsum_j + 1/rowsum_j) + eps).

Since every sampled voxel index lives in [0, 512), cs is a gather of the
512x512 Gram matrix G_b = t2n^T @ t1n.  With E_b = exp(G_b/bw) and c_p[s] the
multiplicity of voxel s in patch p:

    rowsum_j = (E_b @ c_p)[i_j]        colsum_j = (E_b^T @ c_p)[i_j]
    pos_j    = diag(E_b)[i_j]

so the whole (P,B,K,K) tensor never exists:

    loss = -1/(P*B*K) * sum_{b,p,s} c_p[s] *
           log(0.5*diagE_b[s]*(1/CS_b[s,p] + 1/RS_b[s,p]) + eps)

E is stored with a constant exponent offset (E' = E*e^-OFF, fp8e4m3): the
offset cancels in diag/RS and diag/CS, keeping the formula unchanged while
fitting e4m3 range.

Sharding: 8 cores = 2 batches x 4 patch-quarters; per-core partial sums are
returned as a (128,4) tile and summed on the host (no collectives).

Precision: Gram operands are fp8e4m3 (DoubleRow perf mode: 256-deep
contraction per matmul at half cycles/row); norms come from squares of the
same fp8 values (self-consistent normalization); E/E^T are fp8 with the
exponent offset; accumulation, exp/log and the loss reduction stay fp32.
Measured ~5e-5 rel err vs the fp32 reference.

Implementation notes:
- inputs packed into two row-contiguous DRAM tensors (fp8 / bf16) so the
  whole input is 2 dma_starts with one ~2KB descriptor per partition row
  instead of ~1300 1KB descriptors; DMA lands while the engines boot.
- t2-norms are produced directly in column form (per-partition sums via
  small matmuls) to feed exp's per-partition scale without transposes.
- E^T via fp8 PE transposes (stride-2 PSUM writes), PSUM->SBUF copies split
  between ACT and DVE; squares split between DVE and GpSimd.
- CS is accumulated per-t into fresh PSUM tiles (hw PSUM accumulation groups
  must be back-to-back matmuls) and summed on DVE; RS groups stay contiguous.
"""

import math
import os

import ml_dtypes
import numpy as np

import concourse.bacc as bacc
import concourse.tile as tile
from concourse import hw_specs, mybir
from concourse.bass_utils import run_bass_kernel_spmd

# Pin every ACTIVATE to the one table set that holds ln+exp+square+copy, so
# the kernel pays a single ACT_TABLE_LOAD instead of ping-ponging between the
# per-function default sets.
_PIN_SET = "natural_log_exp_and_others"
_orig_get_tables = hw_specs.get_activation_tables


def _pinned_tables(arch):
    tabs = _orig_get_tables(arch)
    return {k: (v if k == _PIN_SET else set()) for k, v in tabs.items()}


bacc.get_activation_tables = _pinned_tables

B, C, S = 2, 256, 512
P, K = 128, 512
BW = 0.05
EPS = 1e-5
N_CORES = 8
PQ = P // 4  # patches per core (32)
EOFF = 1.5  # exponent offset: E' = exp(cs/bw - EOFF); cancels in pos/sum
F32 = mybir.dt.float32
BF16 = mybir.dt.bfloat16
FP8 = mybir.dt.float8e4
DR = mybir.MatmulPerfMode.DoubleRow

PACK = os.environ.get("K_NOPACK") != "1"  # packed 2-tensor input DMA
# GpSimd tensor_tensor is ~2.4x slower than its cost model claims, which also
# misleads the tile scheduler -> keep both squares on DVE.
SQ2_GP = os.environ.get("K_SQ2GP") == "1"
HIPRI = os.environ.get("K_NOHIPRI") != "1"  # norm chain at high priority

# fp8 group layout (bytes per partition row):
#   fx 0:1024 | fy 1024:2048 | ident8 2048:2176
G8_W = 2176
# bf16 group layout (elements per partition row):
#   sq-src? no: cnt 0:128 | ident 128:256 | identh 256:384
G16_W = 384


def _build_program():
    nc = bacc.Bacc("TRN2", target_bir_lowering=False, debug=False, num_devices=N_CORES)

    if PACK:
        grp8 = nc.dram_tensor("grp8", [128, G8_W], FP8, kind="ExternalInput")
        grp16 = nc.dram_tensor("grp16", [128, G16_W], BF16, kind="ExternalInput")
    else:
        fx8 = nc.dram_tensor("fx8", [128, 2, S], FP8, kind="ExternalInput")
        fy8 = nc.dram_tensor("fy8", [128, 2, S], FP8, kind="ExternalInput")
        cntp = nc.dram_tensor("cntp", [128, 128], BF16, kind="ExternalInput")
        identd = nc.dram_tensor("identd", [128, 128], BF16, kind="ExternalInput")
        identd8 = nc.dram_tensor("identd8", [128, 128], FP8, kind="ExternalInput")
        identdh = nc.dram_tensor("identdh", [128, 128], BF16, kind="ExternalInput")
    partial = nc.dram_tensor("partial", [128, 4], F32, kind="ExternalOutput")

    with tile.TileContext(nc) as tc:
        with (
            tc.tile_pool(name="const", bufs=1) as const,
            tc.tile_pool(name="feat", bufs=1) as featp,
            tc.tile_pool(name="big", bufs=1) as big,
            tc.tile_pool(name="tmp", bufs=2) as tmp,
            tc.tile_pool(name="small", bufs=2) as small,
            tc.tile_pool(name="ps_g", bufs=2, space="PSUM") as ps_g,
            tc.tile_pool(name="ps_t", bufs=2, space="PSUM") as ps_t,
            tc.tile_pool(name="ps_cs", bufs=1, space="PSUM") as ps_cs,
            tc.tile_pool(name="ps_misc", bufs=1, space="PSUM") as ps_misc,
        ):
            # ---- input DMAs first: land while the engines boot ----
            if PACK:
                t8 = featp.tile([128, G8_W], FP8, name="t8", tag="t8")
                t16 = const.tile([128, G16_W], BF16, name="t16", tag="t16")
                # fy half first: it gates the longer norm chain (sq1 -> inv1)
                nc.sync.dma_start(out=t8[:, 1024:2176], in_=grp8[:, 1024:2176])
                nc.sync.dma_start(out=t8[:, 0:1024], in_=grp8[:, 0:1024])
                nc.sync.dma_start(out=t16, in_=grp16[:, :])
                fx = t8[:, 0:1024].rearrange("p (i s) -> p i s", i=2)
                fy = t8[:, 1024:2048].rearrange("p (i s) -> p i s", i=2)
                ident8 = t8[:, 2048:2176]
                cnt_all = t16[:, 0:128]
                ident = t16[:, 128:256]
                identh = t16[:, 256:384]
            else:
                fx = featp.tile([128, 2, S], FP8, name="fx", tag="fx")
                fy = featp.tile([128, 2, S], FP8, name="fy", tag="fy")
                cnt_all = const.tile([128, 128], BF16, name="cnt_all", tag="cnt_all")
                ident = const.tile([128, 128], BF16, name="ident", tag="ident")
                ident8 = const.tile([128, 128], FP8, name="ident8", tag="ident8")
                identh = const.tile([128, 128], BF16, name="identh", tag="identh")
                nc.sync.dma_start(out=fx, in_=fx8[:, :, :])
                nc.sync.dma_start(out=fy, in_=fy8[:, :, :])
                nc.sync.dma_start(out=cnt_all, in_=cntp[:, :])
                nc.sync.dma_start(out=ident, in_=identd[:, :])
                nc.sync.dma_start(out=ident8, in_=identd8[:, :])
                nc.sync.dma_start(out=identh, in_=identdh[:, :])
            cnt = [cnt_all[:, PQ * t : PQ * (t + 1)] for t in range(4)]

            ones_col = const.tile([128, 1], BF16, name="ones_col", tag="ocb")
            nc.vector.memset(ones_col, 1.0)
            ones_row = const.tile([1, 128], BF16, name="ones_row", tag="ones_row")
            nc.vector.memset(ones_row, 1.0)
            eps_col = const.tile([128, 1], F32, name="eps_col", tag="eps_col")
            nc.vector.memset(eps_col, EPS)
            ln_ibw_col = const.tile([128, 1], F32, name="ln_ibw_col", tag="lbc")
            nc.vector.memset(ln_ibw_col, math.log(1.0 / BW))
            off_col = const.tile([128, 1], F32, name="off_col", tag="off_col")
            nc.vector.memset(off_col, -EOFF)

            # ---- squares from fp8 features (bf16 out); sq1 gates the longer
            # inv1 chain -> DVE; sq2 in parallel on GpSimd ----
            import contextlib

            hp = tc.high_priority if HIPRI else contextlib.nullcontext

            sq1 = tmp.tile([128, 2, S], BF16, name="sq1", tag="sq1")
            with hp():
                nc.vector.tensor_tensor(
                    out=sq1, in0=fy, in1=fy, op=mybir.AluOpType.mult
                )
            sq2 = tmp.tile([128, 2, S], BF16, name="sq2", tag="sq2")
            if SQ2_GP:
                nc.gpsimd.tensor_tensor(
                    out=sq2, in0=fx, in1=fx, op=mybir.AluOpType.mult
                )
            else:
                nc.vector.tensor_tensor(
                    out=sq2, in0=fx, in1=fx, op=mybir.AluOpType.mult
                )

            # ---- PE: ss1 row, ss2 cols ----
            ss1_ps = ps_misc.tile([1, S], F32, name="ss1_ps", tag="ss1_ps")
            with hp():
                for i in range(2):
                    nc.tensor.matmul(
                        out=ss1_ps, lhsT=ones_col, rhs=sq1[:, i, :],
                        start=(i == 0), stop=(i == 1),
                    )
            # one PSUM bank: rs (cols 0:128), ss2 (128:132)
            mega_ps = ps_misc.tile([128, 132], F32, name="mega_ps", tag="mega_ps")
            ss2c_ps = mega_ps[:, 128:132]
            rs_ps = mega_ps[:, 0:128]
            for m in range(4):
                msl = slice(128 * m, 128 * (m + 1))
                for i in range(2):
                    nc.tensor.matmul(
                        out=ss2c_ps[:, m : m + 1], lhsT=sq2[:, i, msl],
                        rhs=ones_col, start=(i == 0), stop=(i == 1),
                    )

            # Gram: one DoubleRow matmul per 128-row block (contraction 256).
            # Only the first two are emitted here; the last two go after the
            # bc matmul so the scheduler doesn't slot them ahead of it (bc
            # gates exp0 via the gsc multiply).
            def emit_gram(m):
                gp = ps_g.tile([128, S], F32, name=f"g_ps{m}", tag="g_ps")
                nc.tensor.matmul(
                    out=gp, lhsT=fx[:, :, 128 * m : 128 * (m + 1)], rhs=fy,
                    perf_mode=DR, start=True, stop=True,
                )
                return gp

            g_ps = [emit_gram(0), emit_gram(1)]

            # ---- ACT: norms ----
            lns1 = small.tile([1, S], F32, name="lns1", tag="lns1")
            inv1_row = small.tile([1, S], BF16, name="inv1_row", tag="inv1_row")
            with hp():
                nc.scalar.activation(
                    out=lns1, in_=ss1_ps, func=mybir.ActivationFunctionType.Ln
                )
                nc.scalar.activation(
                    out=inv1_row, in_=lns1,
                    func=mybir.ActivationFunctionType.Exp, scale=-0.5,
                )
            lnc2 = small.tile([128, 4], F32, name="lnc2", tag="lnc2")
            nc.scalar.activation(
                out=lnc2, in_=ss2c_ps, func=mybir.ActivationFunctionType.Ln
            )
            inv2bw = small.tile([128, 4], F32, name="inv2bw", tag="inv2bw")
            nc.scalar.activation(
                out=inv2bw, in_=lnc2,
                func=mybir.ActivationFunctionType.Exp,
                scale=-0.5, bias=ln_ibw_col,
            )

            # bc[p, s] = inv1[s] broadcast (PE outer product with ones).
            # bc stays in PSUM; instead the Gram tiles are copied to SBUF on
            # ACT's idle window, so the gsc multiply (one PSUM operand max)
            # runs right after the bc matmul with no serial CAST in between.
            bc_ps = ps_misc.tile([128, S], F32, name="bc_ps", tag="bc_ps")
            with hp():
                nc.tensor.matmul(out=bc_ps, lhsT=ones_row, rhs=inv1_row)

            g_sb = [
                big.tile([128, S], F32, name=f"g_sb{m}", tag=f"g_sb{m}")
                for m in range(4)
            ]
            # split between the two PSUM-capable engines' idle windows
            nc.scalar.activation(
                out=g_sb[0], in_=g_ps[0], func=mybir.ActivationFunctionType.Copy
            )
            nc.scalar.activation(
                out=g_sb[1], in_=g_ps[1], func=mybir.ActivationFunctionType.Copy
            )
            g_ps.append(emit_gram(2))
            g_ps.append(emit_gram(3))
            nc.vector.tensor_copy(out=g_sb[2], in_=g_ps[2])
            nc.vector.tensor_copy(out=g_sb[3], in_=g_ps[3])

            # ---- per-tile: col-scale (DVE), exp (ACT, fp8 out) ----
            e = [
                big.tile([128, S], FP8, name=f"e_{m}", tag=f"e_{m}")
                for m in range(4)
            ]
            for m in range(4):
                g = tmp.tile([128, S], F32, name=f"gsc{m}", tag="gsc")
                nc.vector.tensor_tensor(
                    out=g, in0=g_sb[m], in1=bc_ps, op=mybir.AluOpType.mult
                )
                nc.scalar.activation(
                    out=e[m], in_=g,
                    func=mybir.ActivationFunctionType.Exp,
                    scale=inv2bw[:, m : m + 1], bias=off_col,
                )

            # ---- per-m: transposes -> etm (fp8), CS t-pass, RS ----
            etm = [
                big.tile([128, S], FP8, name=f"etm_{m}", tag=f"etm_{m}")
                for m in range(4)
            ]
            dcol = small.tile([128, 4], F32, name="dcol", tag="dcol")
            cs_acc = small.tile([128, 128], F32, name="cs_acc", tag="cs_acc")

            def emit_transposes(m):
                # fp8 transpose writes PSUM with element step 2: stage in a
                # [128, S, 2] tile and use the stride-2 view as the output.
                et_full = ps_t.tile([128, S, 2], FP8, name=f"et_ps{m}", tag="et_ps")
                et_ps = et_full[:, :, 0]
                for a in range(4):
                    nc.tensor.transpose(
                        out=et_ps[:, 128 * a : 128 * (a + 1)],
                        in_=e[m][:, 128 * a : 128 * (a + 1)],
                        identity=ident8,
                    )
                # PSUM->SBUF move; GpSimd has no PSUM access, so alternate
                # the two engines that do.
                if m % 2 == 0:
                    nc.scalar.activation(
                        out=etm[m], in_=et_ps,
                        func=mybir.ActivationFunctionType.Copy,
                    )
                else:
                    nc.vector.tensor_copy(out=etm[m], in_=et_ps)

            # hw PSUM accumulation groups must be back-to-back matmuls, and
            # the CS t-passes interleave with transposes/RS: accumulate each
            # t-pass into a fresh PSUM tile and sum on DVE.
            def emit_cs_pass(t):
                ctp = ps_cs.tile([128, 128], F32, name=f"cs_t{t}", tag="cs_t")
                for mp in range(4):
                    nc.tensor.matmul(
                        out=ctp[:, PQ * mp : PQ * (mp + 1)],
                        lhsT=e[t][:, 128 * mp : 128 * (mp + 1)],
                        rhs=cnt[t], start=True, stop=True,
                    )
                if t == 0:
                    nc.vector.tensor_copy(out=cs_acc, in_=ctp)
                else:
                    nc.vector.tensor_tensor(
                        out=cs_acc, in0=cs_acc, in1=ctp, op=mybir.AluOpType.add
                    )

            def emit_rs(m):
                for a in range(4):
                    nc.tensor.matmul(
                        out=rs_ps[:, PQ * m : PQ * (m + 1)],
                        lhsT=etm[m][:, 128 * a : 128 * (a + 1)],
                        rhs=cnt[a], start=(a == 0), stop=(a == 3),
                    )

            def emit_dcol(m):
                # dcol[:, m] = 0.5 * diag(E')[msl] via masked row-sum.
                # All-SBUF, so it can run on the otherwise-idle GpSimd,
                # keeping DVE free for the gsc/copy/cs stream.
                scr = tmp.tile([128, 128], BF16, name=f"scr{m}", tag="scr")
                nc.vector.tensor_tensor(
                    out=scr, in0=e[m][:, 128 * m : 128 * (m + 1)], in1=identh,
                    op=mybir.AluOpType.mult,
                )
                nc.vector.tensor_reduce(
                    out=dcol[:, m : m + 1], in_=scr,
                    axis=mybir.AxisListType.X, op=mybir.AluOpType.add,
                )

            # per-m groups; RS(m) is delayed one group so the etm copy is done
            emit_transposes(0)
            emit_cs_pass(0)
            emit_dcol(0)
            emit_transposes(1)
            emit_cs_pass(1)
            emit_dcol(1)
            emit_rs(0)
            emit_transposes(2)
            emit_cs_pass(2)
            emit_dcol(2)
            emit_rs(1)
            emit_transposes(3)
            emit_cs_pass(3)
            emit_dcol(3)
            emit_rs(2)
            emit_rs(3)

            # ---- tail: sum_c c * ln(0.5*d*(1/RS+1/CS) + eps) ----
            # full-tile ops where possible (fewer instructions = shorter
            # dependency/semaphore tail); LN stays per-m for the d-scale.
            rinv = small.tile([128, 128], F32, name="rinv", tag="rinv")
            cinv = small.tile([128, 128], F32, name="cinv", tag="cinv")
            # RS/CS are sums of positive e-values (no 0/inf/denorm);
            # ~18-bit reciprocal is far inside the error budget
            nc.vector.reciprocal_approx_fast(out=rinv, in_=rs_ps)
            nc.vector.reciprocal_approx_fast(out=cinv, in_=cs_acc)
            ssum = small.tile([128, 128], F32, name="ssum", tag="ssum")
            nc.vector.tensor_tensor(
                out=ssum, in0=rinv, in1=cinv, op=mybir.AluOpType.add
            )
            gl = small.tile([128, 128], F32, name="gl", tag="gl")
            for m in range(4):
                mcol = slice(PQ * m, PQ * (m + 1))
                nc.scalar.activation(
                    out=gl[:, mcol], in_=ssum[:, mcol],
                    func=mybir.ActivationFunctionType.Ln,
                    scale=dcol[:, m : m + 1], bias=eps_col,
                )
            wgl = small.tile([128, 128], F32, name="wgl", tag="wgl")
            nc.vector.tensor_tensor(
                out=wgl, in0=gl, in1=cnt_all, op=mybir.AluOpType.mult
            )
            # acc kept at 4 columns: a [128,1] f32 output makes 4-byte DMA
            # descriptors, which lands the NEFF in a much slower teardown
            # path (~+5us) — 16-byte rows avoid it.
            acc = small.tile([128, 4], F32, name="acc", tag="acc")
            for m in range(4):
                nc.vector.tensor_reduce(
                    out=acc[:, m : m + 1],
                    in_=wgl[:, PQ * m : PQ * (m + 1)],
                    axis=mybir.AxisListType.X, op=mybir.AluOpType.add,
                )
            nc.sync.dma_start(out=partial[:, :], in_=acc)

    nc.compile()
    return nc


_NC = None


def _pack_inputs(t2, t1, idx):
    counts = np.zeros((P, S), np.float32)
    np.add.at(counts, (np.arange(P)[:, None], idx), 1.0)
    identf = np.eye(128, dtype=np.float32)

    in_maps = []
    for core in range(N_CORES):
        b, q = divmod(core, 4)
        f2i = np.ascontiguousarray(
            t2[b].reshape(2, 128, S).transpose(1, 0, 2).reshape(128, 1024)
        )
        f1i = np.ascontiguousarray(
            t1[b].reshape(2, 128, S).transpose(1, 0, 2).reshape(128, 1024)
        )
        cq = np.ascontiguousarray(
            counts[PQ * q : PQ * (q + 1)]
            .T.reshape(4, 128, PQ)
            .transpose(1, 0, 2)
            .reshape(128, 128)
        )
        if PACK:
            grp8 = np.concatenate([f2i, f1i, identf], axis=1).astype(
                ml_dtypes.float8_e4m3fn
            )
            grp16 = np.concatenate([cq, identf, 0.5 * identf], axis=1).astype(
                ml_dtypes.bfloat16
            )
            in_maps.append({"grp8": grp8, "grp16": grp16})
        else:
            in_maps.append(
                {
                    "fx8": f2i.reshape(128, 2, S).astype(ml_dtypes.float8_e4m3fn),
                    "fy8": f1i.reshape(128, 2, S).astype(ml_dtypes.float8_e4m3fn),
                    "cntp": cq.astype(ml_dtypes.bfloat16),
                    "identd": identf.astype(ml_dtypes.bfloat16),
                    "identd8": identf.astype(ml_dtypes.float8_e4m3fn),
                    "identdh": (0.5 * identf).astype(ml_dtypes.bfloat16),
                }
            )
    return in_maps


def _run(t2_feat, t1_feat, idx, trace=False, trace_kwargs=None):
    global _NC
    if _NC is None:
        _NC = _build_program()

    t2 = np.ascontiguousarray(np.asarray(t2_feat, np.float32).reshape(B, C, S))
    t1 = np.ascontiguousarray(np.asarray(t1_feat, np.float32).reshape(B, C, S))
    idx = np.asarray(idx)
    in_maps = _pack_inputs(t2, t1, idx)

    kwargs = {}
    if trace:
        kwargs = dict(trace=True, trace_kwargs=trace_kwargs or {})
    res = run_bass_kernel_spmd(_NC, in_maps, core_ids=list(range(N_CORES)), **kwargs)
    total = sum(r["partial"].sum(dtype=np.float64) for r in res.results)
    loss = -total / (P * B * K)
    return np.array(loss, dtype=np.float32), res


def kernel(t2_feat, t1_feat, idx):
    out, _ = _run(t2_feat, t1_feat, idx)
    return out
